# revision 14
# baseline (speedup 1.0000x reference)
"""Segment-max aggregation of gathered embedding rows (NodeMaxAggregator).

out[n, :] = max_{e : segment_ids[e]==n} table[hyperedge_ids[e], :]

Strategy (8 NeuronCores, data-parallel over node segments):
  - Nodes are globally sorted by their per-window entry-count vector and
    split contiguously across the 8 cores (host applies the inverse
    permutation to the result at the end).
  - The table is re-laid-out into windows of WROWS rows plus one -inf pad
    row each, so every row is addressable by a window-local int16 index,
    enabling bulk InstDMAGatherAnt (SWDGE) gathers on 4 queues.
  - Per tile of 128 nodes and per window m, a rectangle of 128 x K[t,m]
    indices is built (K = max per-node count in the tile; slots beyond a
    node's count repeat one of its own entries, or the window's -inf row
    if it has none). One dma_gather per (tile-group, window) fetches all
    rectangles of the group.
  - DVE merges each tile's window spans in place (pairwise prefix max,
    then a halving tree) down to one [128, D] column -> DMA'd out.
  - Each core gets its own program (schedules are data-dependent and
    differ per core); all 8 are dispatched asynchronously, one per
    NeuronCore, and executed concurrently.
"""

import math
from contextlib import ExitStack

import numpy as np

P = 128
N_CORES = 8
WROWS = 28572          # real rows per window (+1 pad row = 28573 <= int16 max)
GROUP_COL_CAP = 140    # max gathered columns resident per tile-group
NQ = 4                 # SWDGE queues

LAST_RESULT = None     # optional BassKernelResults-like info for tests
LAST_EXEC_WALL_S = None


# ---------------------------------------------------------------- planning

def _pad_entries(hyperedge_ids, segment_ids, n_nodes):
    e_total = hyperedge_ids.shape[0]
    counts = np.bincount(segment_ids, minlength=n_nodes).astype(np.int64)
    k = max(int(counts.max()), 1) if e_total else 1
    starts = np.zeros(n_nodes, dtype=np.int64)
    np.cumsum(counts[:-1], out=starts[1:])
    rank = np.arange(e_total, dtype=np.int64) - starts[segment_ids]
    idx = np.full((n_nodes, k), -1, dtype=np.int64)
    idx[segment_ids, rank] = hyperedge_ids
    return idx, k


def _core_schedule(rows, nwin):
    """Build the gather/reduce schedule for one core.

    rows: [n_pc_pad, k] global row ids, -1 for unused slots.
    Returns (groups, idx_np[P, W] int16, max_cols, n_tiles).
    groups: list of dicts:
      gathers: list of (window, num_idxs, word_off, col_base)
      tile_spans: {local tile -> [(col, width), ...]}
      tiles: local tile indices
    """
    n_pc_pad, k = rows.shape
    n_tiles = n_pc_pad // P
    valid = rows >= 0
    wof = np.where(valid, rows // WROWS, -1)
    loc = np.where(valid, rows % WROWS, 0)

    # per-window padded value blocks for all nodes: [n_pc_pad, Kmax_m]
    blocks = []           # per window: [n_pc_pad, Km_global] int16
    cnts = np.zeros((n_pc_pad, nwin), np.int32)
    for m in range(nwin):
        mask = wof == m
        cnt = mask.sum(axis=1)
        cnts[:, m] = cnt
        km = int(cnt.max()) if n_pc_pad else 0
        if km == 0:
            blocks.append(None)
            continue
        order = np.argsort(~mask, axis=1, kind="stable")[:, :km]
        vals = np.take_along_axis(np.where(mask, loc, 0), order, axis=1)
        first = np.where(cnt > 0, vals[:, 0], WROWS)
        jj = np.arange(km)[None, :]
        blk = np.where(jj < cnt[:, None], vals, first[:, None])
        blocks.append(blk.astype(np.int16))

    # per-tile per-window K
    Ks = cnts.reshape(n_tiles, P, nwin).max(axis=1)  # [n_tiles, nwin]
    tile_cols = Ks.sum(axis=1)

    # group consecutive tiles under the column cap
    groups_tiles = []
    cur, cur_cols = [], 0
    for t in range(n_tiles):
        tc = int(tile_cols[t])
        if cur and cur_cols + tc > GROUP_COL_CAP:
            groups_tiles.append(cur)
            cur, cur_cols = [], 0
        cur.append(t)
        cur_cols += tc
    if cur:
        groups_tiles.append(cur)

    idx_blocks = []
    word_off = 0
    groups = []
    max_cols = 0
    for g in groups_tiles:
        col = 0
        gathers = []
        tile_spans = {t: [] for t in g}
        for m in range(nwin):
            parts = []
            cbase = col
            for t in g:
                km = int(Ks[t, m])
                if km == 0:
                    continue
                blk = blocks[m][t * P:(t + 1) * P, :km]   # [P, km]
                tile_spans[t].append((col, km))
                col += km
                parts.append(blk.T.reshape(-1))            # j-major flat
            if not parts:
                continue
            flat = np.concatenate(parts)
            num_idxs = len(flat)
            wblk = flat.reshape(-1, 16).T                  # [16, num/16]
            idx_blocks.append(wblk)
            gathers.append((m, num_idxs, word_off, cbase))
            word_off += wblk.shape[1]
        groups.append({"tiles": g, "gathers": gathers, "tile_spans": tile_spans,
                       "cols": col})
        max_cols = max(max_cols, col)

    idx16 = (np.concatenate(idx_blocks, axis=1) if idx_blocks
             else np.zeros((16, 16), np.int16))
    pad = (-idx16.shape[1]) % 16
    if pad:
        idx16 = np.concatenate([idx16, np.zeros((16, pad), np.int16)], axis=1)
    idx_np = np.tile(idx16, (8, 1)).astype(np.int16)       # replicate x8
    return groups, idx_np, max_cols, n_tiles


def _reduce_spans_ops(spans):
    """(dst_off, src_off, width) in-place max ops collapsing spans to 1 col."""
    ops = []
    spans = sorted(spans, key=lambda s: -s[1])
    while len(spans) > 1:
        o1, w1 = spans[0]
        o2, w2 = spans[1]
        ops.append((o1, o2, w2))
        spans = [(o1, w1)] + spans[2:]
    o, w = spans[0]
    while w > 1:
        if w % 2:
            ops.append((o, o + w - 1, 1))
        half = w // 2
        ops.append((o, o + half, half))
        w = half
    return ops, o


# ---------------------------------------------------------------- device

def _build_program(nwin, aug_rows, d, n_pc_pad, groups, idx_cols, max_cols):
    import concourse.mybir as mybir
    import concourse.tile as tile
    from concourse import bacc

    f32, i16 = mybir.dt.float32, mybir.dt.int16

    nc = bacc.Bacc(None, target_bir_lowering=False, num_swdge_queues=NQ)
    table = nc.dram_tensor("table", [aug_rows, d], f32, kind="ExternalInput")
    idx = nc.dram_tensor("idx", [P, idx_cols], i16, kind="ExternalInput")
    out = nc.dram_tensor("out", [n_pc_pad, d], f32, kind="ExternalOutput")
    out_r = out.rearrange("(t p) (one d) -> t p one d", p=P, one=1)

    qrr = 0
    with ExitStack() as ctx:
        tc = ctx.enter_context(tile.TileContext(nc))
        ip = ctx.enter_context(tc.tile_pool(name="ip", bufs=2))
        gp = ctx.enter_context(tc.tile_pool(name="gp", bufs=2))

        for g in groups:
            if not g["gathers"]:
                continue
            w0 = g["gathers"][0][2]
            w1 = g["gathers"][-1][2] + (g["gathers"][-1][1] + 15) // 16
            gidx = ip.tile([P, w1 - w0], i16, tag="gidx", name="gidx")
            nc.sync.dma_start(gidx[:], idx[:, w0:w1])

            gt = gp.tile([P, g["cols"], d], f32, tag="gt", name="gt")
            for (m, num_idxs, woff, cbase) in g["gathers"]:
                ncols = num_idxs // P
                nc.gpsimd.dma_gather(
                    out_ap=gt[:, cbase:cbase + ncols, :],
                    in_ap=table[m * (WROWS + 1):(m + 1) * (WROWS + 1), :],
                    idxs_ap=gidx[:, woff - w0: woff - w0 + num_idxs // 16],
                    num_idxs=num_idxs,
                    num_idxs_reg=num_idxs,
                    elem_size=d,
                    single_packet=False,
                    queue_num=qrr % NQ,
                )
                qrr += 1

            for t in g["tiles"]:
                spans = g["tile_spans"][t]
                if not spans:
                    continue
                ops, fin = _reduce_spans_ops(spans)
                for (do, so, w) in ops:
                    nc.vector.tensor_max(
                        gt[:, do:do + w, :], gt[:, do:do + w, :],
                        gt[:, so:so + w, :])
                nc.sync.dma_start(out_r[t], gt[:, fin:fin + 1, :])

    nc.finalize()
    return nc


# ---------------------------------------------------------------- runner

def _run_programs(ncs, in_maps):
    """Dispatch one single-core program per device, concurrently."""
    import time

    import jax

    import concourse.mybir as mybir
    from concourse import bass2jax

    bass2jax.install_neuronx_cc_hook()
    devices = jax.devices()
    assert len(devices) >= len(ncs)

    compiled = []
    for c, nc in enumerate(ncs):
        assert nc.dbg_addr is None
        partition_name = (nc.partition_id_tensor.name
                          if nc.partition_id_tensor else None)
        in_names, out_names, out_avals, zero_outs = [], [], [], []
        for alloc in nc.m.functions[0].allocations:
            if not isinstance(alloc, mybir.MemoryLocationSet):
                continue
            name = alloc.memorylocations[0].name
            if alloc.kind == "ExternalInput":
                if name != partition_name:
                    in_names.append(name)
            elif alloc.kind == "ExternalOutput":
                shape = tuple(alloc.tensor_shape)
                dtype = mybir.dt.np(alloc.dtype)
                out_names.append(name)
                out_avals.append(jax.core.ShapedArray(shape, dtype))
                zero_outs.append(np.zeros(shape, dtype))

        all_names = list(in_names) + list(out_names)
        if partition_name is not None:
            all_names.append(partition_name)

        def make_body(nc=nc, out_avals=tuple(out_avals),
                      all_names=tuple(all_names), out_names=tuple(out_names),
                      partition_name=partition_name):
            def _body(*args):
                operands = list(args)
                if partition_name is not None:
                    operands.append(bass2jax.partition_id_tensor())
                return tuple(bass2jax._bass_exec_p.bind(
                    *operands,
                    out_avals=out_avals,
                    in_names=all_names,
                    out_names=out_names,
                    lowering_input_output_aliases=(),
                    sim_require_finite=True,
                    sim_require_nnan=True,
                    nc=nc,
                ))
            return _body

        donate = tuple(range(len(in_names), len(in_names) + len(out_names)))
        fn = jax.jit(make_body(), donate_argnums=donate, keep_unused=True)
        compiled.append((fn, in_names, out_names, zero_outs))

    # stage inputs on each device
    def stage(c):
        fn, in_names, out_names, zero_outs = compiled[c]
        dev = devices[c]
        args = [jax.device_put(np.asarray(in_maps[c][n]), dev)
                for n in in_names]
        args += [jax.device_put(z, dev) for z in zero_outs]
        return args

    # compile (first dispatch) in threads so the neuronx-cc subprocesses
    # overlap; donated buffers are consumed, so re-stage for timing runs
    import concurrent.futures as cf

    def compile_one(c):
        fn = compiled[c][0]
        args = stage(c)
        outs = fn(*args)
        jax.block_until_ready(outs)
        return outs

    with cf.ThreadPoolExecutor(max_workers=len(ncs)) as ex:
        first = list(ex.map(compile_one, range(len(ncs))))

    # timed run: dispatch all, then block
    args_all = [stage(c) for c in range(len(ncs))]
    t0 = time.time()
    outs_all = [compiled[c][0](*args_all[c]) for c in range(len(ncs))]
    for o in outs_all:
        jax.block_until_ready(o)
    wall = time.time() - t0

    global LAST_EXEC_WALL_S
    LAST_EXEC_WALL_S = wall

    results = []
    for c in range(len(ncs)):
        out_names = compiled[c][2]
        results.append({n: np.asarray(a)
                        for n, a in zip(out_names, outs_all[c])})
    return results


# ---------------------------------------------------------------- driver

def kernel(table, hyperedge_ids, segment_ids, n_nodes):
    n_nodes = int(n_nodes)
    table = np.ascontiguousarray(np.asarray(table, dtype=np.float32))
    hyperedge_ids = np.asarray(hyperedge_ids)
    segment_ids = np.asarray(segment_ids)
    n_rows, d = table.shape
    assert (d * 4) % 256 == 0

    nwin = max(1, math.ceil(n_rows / WROWS))
    aug_rows = nwin * (WROWS + 1)
    neg = np.float32(-3.0e38)
    table_aug = np.full((aug_rows, d), neg, np.float32)
    for m in range(nwin):
        lo = m * WROWS
        hi = min(lo + WROWS, n_rows)
        dst = m * (WROWS + 1)
        table_aug[dst:dst + hi - lo] = table[lo:hi]

    idx_padded, k = _pad_entries(hyperedge_ids, segment_ids, n_nodes)

    # global sort of nodes by per-window count vector
    valid = idx_padded >= 0
    wof = np.where(valid, idx_padded // WROWS, -1)
    wcounts = np.zeros((n_nodes, nwin), np.int32)
    for m in range(nwin):
        wcounts[:, m] = (wof == m).sum(axis=1)
    order = np.lexsort(tuple(wcounts[:, m] for m in reversed(range(nwin))))

    n_per_core = math.ceil(n_nodes / N_CORES)
    n_tiles = math.ceil(n_per_core / P)
    n_pc_pad = n_tiles * P

    ncs, in_maps = [], []
    for c in range(N_CORES):
        lo = min(c * n_per_core, n_nodes)
        hi = min(lo + n_per_core, n_nodes)
        nodes = order[lo:hi]
        rows = np.full((n_pc_pad, k), -1, dtype=np.int64)
        rows[: hi - lo] = idx_padded[nodes]
        groups, idx_np, max_cols, nt = _core_schedule(rows, nwin)
        nc = _build_program(nwin, aug_rows, d, n_pc_pad, groups,
                            idx_np.shape[1], max_cols)
        ncs.append(nc)
        in_maps.append({"table": table_aug, "idx": idx_np})

    results = _run_programs(ncs, in_maps)

    out_full = np.empty((n_nodes, d), np.float32)
    for c in range(N_CORES):
        lo = min(c * n_per_core, n_nodes)
        hi = min(lo + n_per_core, n_nodes)
        out_full[order[lo:hi]] = results[c]["out"][: hi - lo]
    return out_full
